# revision 1
# baseline (speedup 1.0000x reference)
"""Trainium2 Bass kernel for Attention_concat (separable PAM attention).

Math (per batch b, N = H*W = 4096):
    eq = x^T wq_eff + bq_eff        (wq_eff = Wq^T wq, wq = Wc[:inter])
    ek = x^T wk_eff + bk_eff
    attention[i, j] = (eq[i] + ek[j]) / N          (rank-structured, never built)
    out = v @ attention, v = Wv x + bv
    y = gamma * out + x
      = x + A[c] + Bv[c] * ekn[m]                  (rank-1 over spatial m)
with
    u = x @ 1, t = x @ eqn (eqn = eq - bq_eff), E_nb = wq_eff . u
    s_nb = Wv (t + bq_eff u),  V_nb = Wv u
    g = gamma / N
    Bv = g V_nb + g N bv
    A  = g (s_nb + bk_eff V_nb) + bv (g E_nb + g N (bq_eff + bk_eff))

Precision: the attention term is ~1.5e-4 of |y| (x dominates), so the whole
reduction pipeline runs in bf16 on the PE at full rate; only the final
y = x + psum add uses exact fp32 x.

Sharding: 2 cores per batch; each core receives the full x[b] (column-permuted
so its own half of the spatial positions comes first), computes the global
reductions redundantly, and writes the first 2048 output columns.

Engine split per 1024-column superblock: HWDGE DMA streams x (fp32 for the
core's own half, host-cast bf16 for the other half); ACT fuses the bf16 cast
with the u accumulation (activation Copy + accum_out); PE broadcasts eq
directly into PSUM via a matmul whose stationary operand is wq_eff replicated
along the free dim; DVE fuses the t multiply+reduce in one
scalar_tensor_tensor (accum_out); ek rows land straight in the RC tile for
phase C, where one matmul per 512 block computes Bv⊗ek + A⊗ones and one DVE
add applies x. Dummy matmuls on a memset tile warm the PE HAM clock gate
during the initial DMA wait. Module-level workarounds: this container's
walrus accepts only one sync-wait per instruction, so extra waits are hoisted
onto single-wait NoOps at BIR level, and the Tile tail drain is rebuilt the
same way.
"""

import json as _json

import numpy as np

import concourse.bass as bass
import concourse.bass2jax as _b2j
import concourse.bass_utils as _bu
import concourse.mybir as mybir
import concourse.tile as tile
from concourse.bass_utils import run_bass_kernel_spmd
from concourse.tile_rust import add_dep_helper
from concourse.vector_clock import ScopedClock, VectorClock

B, C, H, W = 4, 256, 64, 64
N = H * W            # 4096
INTER = C // 4       # 64
NCORES = 8
HALF = N // 2        # 2048 output columns per core
SUP = 4              # 1024-column DMA superblocks
F32 = mybir.dt.float32
BF16 = mybir.dt.bfloat16
AX = mybir.AxisListType
OP = mybir.AluOpType
ACTF = mybir.ActivationFunctionType


def _split_multi_waits(bir: dict) -> dict:
    """The nix walrus accepts only ONE sync-wait command per instruction.
    Hoist extra waits onto preceding single-wait NoOps on the same engine
    (sequencers execute in program order, so semantics are unchanged)."""
    ctr = 0
    for fn in bir.get("functions", []):
        for blk in fn.get("blocks", []):
            insts = blk.get("instructions")
            if not insts:
                continue
            out = []
            for inst in insts:
                si = inst.get("sync_info") or {}
                waits = si.get("on_wait") or []
                if len(waits) > 1 and inst.get("engine", "Unassigned") != "Unassigned":
                    for w in waits[:-1]:
                        ctr += 1
                        out.append({
                            "debug": inst.get("debug", 0),
                            "engine": inst["engine"],
                            "ins": [], "outs": [],
                            "name": f"{inst['name']}-ws{ctr}",
                            "opcode": "NoOp",
                            "sync_info": {"on_update": [], "on_wait": [w]},
                        })
                    si["on_wait"] = [waits[-1]]
                out.append(inst)
            blk["instructions"] = out
    return bir


_WAIT_SPLIT_DONE = False


def install_wait_split():
    global _WAIT_SPLIT_DONE
    if _WAIT_SPLIT_DONE:
        return
    orig = _bu.compile_bir_kernel

    def wrapped(bir_json, *a, **kw):
        d = _json.loads(bir_json)
        _split_multi_waits(d)
        return orig(_json.dumps(d).encode(), *a, **kw)

    _bu.compile_bir_kernel = wrapped
    _b2j.compile_bir_kernel = wrapped
    _WAIT_SPLIT_DONE = True


class SplitDrainTileContext(tile.TileContext):
    """Tail fix for the same 1-wait walrus limit: park the global-clock waits
    on single-wait Nops spread across all five engines (they wait in
    parallel), then a wait-free drain + the usual barrier/reset."""

    def _drain_and_barrier(self, tick_clock, wait_clock):
        gc = tick_clock.global_clock
        nprocs = len(gc)
        engines = [self.nc.sync, self.nc.vector, self.nc.scalar,
                   self.nc.gpsimd, self.nc.tensor]
        idx = 0
        for proc in range(nprocs):
            if gc[proc] > 0:
                eng = engines[idx % len(engines)]
                idx += 1
                nop = eng.nop(nofuse=True, hint=f"tail_wait_p{proc}")
                vc = VectorClock([0] * nprocs)
                vc.require_at_least(proc, gc[proc])
                wait_clock.add_sem_waits(nop.ins, ScopedClock({None: vc}))
        self.nc.sync.drain()
        self.nc.all_engine_barrier()
        assert self.sems is not None
        popped = self.nc._tile_sem_poison_stack.pop()
        assert popped is self._sem_poison
        self.nc.clear_and_free_semaphores(list(self.sems.allocated().values()))
        self.nc.all_engine_barrier()


def build_kernel(g: float, bq_eff: float, bk_eff: float):
    """Build the per-core Bass program. g = gamma/N."""
    nc = bass.Bass()
    xin = nc.dram_tensor("xin", [C, HALF], F32, kind="ExternalInput")
    xbh = nc.dram_tensor("xbh", [C, HALF], BF16, kind="ExternalInput")
    wqk = nc.dram_tensor("wqk", [128, 2, 2], BF16, kind="ExternalInput")
    wqrep = nc.dram_tensor("wqrep", [128, 2, 128], BF16, kind="ExternalInput")
    wvt = nc.dram_tensor("wvt", [128, 2, C], BF16, kind="ExternalInput")
    bvrow = nc.dram_tensor("bvrow", [1, C], F32, kind="ExternalInput")
    bvgn = nc.dram_tensor("bvgn", [1, C], F32, kind="ExternalInput")
    yout = nc.dram_tensor("yout", [C, HALF], F32, kind="ExternalOutput")

    with SplitDrainTileContext(nc) as tc:
        with (
            tc.tile_pool(name="persist", bufs=1) as pp,
            tc.tile_pool(name="trash", bufs=4) as tp,
            tc.tile_pool(name="ypool", bufs=4) as yp,
            tc.tile_pool(name="prows", bufs=2, space="PSUM") as prows,
            tc.tile_pool(name="pbig", bufs=3, space="PSUM") as pbig,
        ):
            # --- persistent tiles -------------------------------------------------
            # fp32 x only for the core's own half (exact final add)
            xt = [[pp.tile([128, 1024], F32, tag=f"x{q}_{k}", name=f"x{q}_{k}")
                   for k in range(2)] for q in range(2)]
            xbf = [[pp.tile([128, 1024], BF16, tag=f"xb{q}_{k}", name=f"xb{q}_{k}")
                    for k in range(SUP)] for q in range(2)]
            wqk_sb = pp.tile([128, 2, 2], BF16, tag="wqk")
            wqrep_sb = pp.tile([128, 2, 128], BF16, tag="wqrep")
            wvt_sb = pp.tile([128, 2, C], BF16, tag="wvt")
            bv_sb = pp.tile([1, C], F32, tag="bv")
            bvgn_sb = pp.tile([1, C], F32, tag="bvgn")
            RC = pp.tile([2, HALF], BF16, tag="RC")    # row0 = ek, row1 = ones
            ONES1 = pp.tile([1, HALF], BF16, tag="ONES1")
            AB = pp.tile([2, C], BF16, tag="AB")       # row0 = Bv, row1 = A
            tacc = pp.tile([128, 2, SUP], F32, tag="tacc")
            uacc = pp.tile([128, 2, SUP], F32, tag="uacc")
            t2 = pp.tile([128, 2], F32, tag="t2")
            u2 = pp.tile([128, 2], F32, tag="u2")
            tu = pp.tile([128, 2, 2], F32, tag="tu")
            tub = pp.tile([128, 2, 2], BF16, tag="tub")
            u2b = pp.tile([128, 2], BF16, tag="u2b")
            s_sb = pp.tile([1, C], F32, tag="s_sb")
            v_sb = pp.tile([1, C], F32, tag="v_sb")
            e_sb = pp.tile([1, 1], F32, tag="e_sb")
            sc_sb = pp.tile([1, 1], F32, tag="sc_sb")
            aa = pp.tile([1, C], BF16, tag="aa")
            abv = pp.tile([1, C], BF16, tag="abv")
            tm1 = pp.tile([1, C], F32, tag="tm1")
            tm2 = pp.tile([1, C], F32, tag="tm2")

            nc.gpsimd.memset(ONES1, 1.0)
            # RC row1 = ones (cross-partition row move via DMA)
            nc.sync.dma_start(out=RC[1:2, :], in_=ONES1[0:1, :])
            # first compute block's data goes out before the weights so it
            # finishes the SDMA round-robin soonest
            for q in range(2):
                nc.sync.dma_start(out=xt[q][0],
                                  in_=xin[128 * q:128 * (q + 1), 0:1024])
            nc.sync.dma_start(out=wqk_sb, in_=wqk[:, :, :])
            nc.sync.dma_start(out=wqrep_sb, in_=wqrep[:, :, :])
            nc.sync.dma_start(out=wvt_sb, in_=wvt[:, :, :])
            nc.sync.dma_start(out=bv_sb, in_=bvrow[:, :])
            nc.sync.dma_start(out=bvgn_sb, in_=bvgn[:, :])

            # PE warm-up: dummy matmuls on a memset tile (no DMA dependency)
            # during the DMA wait, so the HAM clock gate reaches 2.4 GHz
            # before the real matmuls arrive.
            wusrc = pp.tile([128, 512], BF16, tag="wusrc")
            nc.vector.memset(wusrc, 0.5)
            for i in range(26):
                wu = pbig.tile([128, 512], F32, tag="big", name=f"wu{i}")
                nc.tensor.matmul(wu, wusrc[:, 0:128], wusrc,
                                 start=True, stop=True)

            # --- phase A: stream x, cast, eqb direct, ek rows, t/u ---------------
            for k in range(SUP):
                for q in range(2):
                    if k < 2:
                        # own half: fp32 load + fused bf16 cast + u accumulate
                        if k > 0:
                            nc.sync.dma_start(
                                out=xt[q][k],
                                in_=xin[128 * q:128 * (q + 1),
                                        1024 * k:1024 * (k + 1)],
                            )
                        nc.scalar.activation(
                            out=xbf[q][k], in_=xt[q][k], func=ACTF.Copy,
                            accum_out=uacc[:, q, k:k + 1],
                        )
                    else:
                        # other half: bf16 straight from the host, u only
                        nc.sync.dma_start(
                            out=xbf[q][k],
                            in_=xbh[128 * q:128 * (q + 1),
                                    1024 * (k - 2):1024 * (k - 1)],
                        )
                        tru = tp.tile([128, 1024], BF16, tag="prod")
                        nc.scalar.activation(
                            out=tru, in_=xbf[q][k], func=ACTF.Copy,
                            accum_out=uacc[:, q, k:k + 1],
                        )
                # eqb direct: lhsT = wq_eff replicated along the free dim, so
                # every output row i gets eq[n] — the partition broadcast comes
                # out of the matmul itself, no eq-row round trip needed.
                eqb = pbig.tile([128, 1024], F32, tag="big")
                for sub in range(2):
                    blk = slice(512 * sub, 512 * (sub + 1))
                    for q in range(2):
                        nc.tensor.matmul(eqb[:, blk], wqrep_sb[:, q, :],
                                         xbf[q][k][:, blk],
                                         start=(q == 0), stop=(q == 1))
                # ek row, only for the core's own output half (k < 2)
                if k < 2:
                    for sub in range(2):
                        blk = slice(512 * sub, 512 * (sub + 1))
                        gcol = slice(1024 * k + 512 * sub,
                                     1024 * k + 512 * sub + 512)
                        ekp = prows.tile([1, 512], F32, tag="qk")
                        for q in range(2):
                            nc.tensor.matmul(ekp, wqk_sb[:, q, 1:2],
                                             xbf[q][k][:, blk],
                                             start=(q == 0), stop=(q == 1))
                        nc.scalar.copy(out=RC[0:1, gcol], in_=ekp)
                for q in range(2):
                    # t-partial: fused multiply+reduce in one DVE pass
                    src = xt[q][k] if k < 2 else xbf[q][k]
                    prod = tp.tile([128, 1024], BF16, tag="prod")
                    nc.vector.scalar_tensor_tensor(
                        out=prod, in0=src, scalar=0.0, in1=eqb,
                        op0=OP.add, op1=OP.mult,
                        accum_out=tacc[:, q, k:k + 1],
                    )

            # --- tail: fold reductions into A/Bv rows ----------------------------
            for q in range(2):
                nc.vector.tensor_reduce(out=t2[:, q:q + 1], in_=tacc[:, q, :],
                                        axis=AX.X, op=OP.add)
                nc.vector.tensor_reduce(out=u2[:, q:q + 1], in_=uacc[:, q, :],
                                        axis=AX.X, op=OP.add)
                # tu[:,q,0] = t + bq_eff*u ; tu[:,q,1] = u
                nc.vector.tensor_scalar(out=tu[:, q, 1:2], in0=u2[:, q:q + 1],
                                        scalar1=bq_eff, scalar2=None, op0=OP.mult)
                nc.vector.tensor_tensor(out=tu[:, q, 0:1], in0=tu[:, q, 1:2],
                                        in1=t2[:, q:q + 1], op=OP.add)
                nc.vector.tensor_copy(out=tu[:, q, 1:2], in_=u2[:, q:q + 1])
                nc.vector.tensor_copy(out=tub[:, q, :], in_=tu[:, q, :])
                nc.vector.tensor_copy(out=u2b[:, q:q + 1], in_=u2[:, q:q + 1])

            ep = prows.tile([1, 1], F32, tag="qk")
            sp = prows.tile([1, C], F32, tag="qk")
            vp = prows.tile([1, C], F32, tag="qk")
            for q in range(2):
                nc.tensor.matmul(ep, u2b[:, q:q + 1], wqk_sb[:, q, 0:1],
                                 start=(q == 0), stop=(q == 1))
                nc.tensor.matmul(sp, tub[:, q, 0:1], wvt_sb[:, q, :],
                                 start=(q == 0), stop=(q == 1))
                nc.tensor.matmul(vp, tub[:, q, 1:2], wvt_sb[:, q, :],
                                 start=(q == 0), stop=(q == 1))
            nc.scalar.copy(out=e_sb, in_=ep)
            # wvt is pre-scaled by g on the host, so sp/vp rows are already
            # s2 = g*s_nb and V2 = g*V_nb.
            nc.scalar.copy(out=s_sb, in_=sp)
            nc.scalar.copy(out=v_sb, in_=vp)

            # sc = g*E_nb + g*N*(bq_eff + bk_eff)
            nc.scalar.activation(out=sc_sb, in_=e_sb, func=ACTF.Copy,
                                 bias=g * N * (bq_eff + bk_eff), scale=g)
            # A = s2 + bk_eff*V2 + bv*sc ; Bv = V2 + g*N*bv (bvgN from host)
            nc.vector.tensor_scalar(out=tm1, in0=v_sb, scalar1=bk_eff,
                                    scalar2=None, op0=OP.mult)
            nc.vector.tensor_tensor(out=tm1, in0=tm1, in1=s_sb, op=OP.add)
            nc.vector.tensor_scalar(out=tm2, in0=bv_sb, scalar1=sc_sb,
                                    scalar2=None, op0=OP.mult)
            nc.vector.tensor_tensor(out=aa, in0=tm1, in1=tm2, op=OP.add)
            nc.vector.tensor_tensor(out=abv, in0=v_sb, in1=bvgn_sb, op=OP.add)
            # AB rows: row0 = Bv (pairs with RC row0 = ek), row1 = A (ones)
            nc.sync.dma_start(out=AB[0:1, :], in_=abv[0:1, :])
            nc.sync.dma_start(out=AB[1:2, :], in_=aa[0:1, :])

            # --- phase C: y = x + A + Bv*ek over own half (first 2048 cols) ------
            for q in range(2):
                for k in range(2):
                    ys = yp.tile([128, 1024], F32, tag="y")
                    yps = pbig.tile([128, 1024], F32, tag="big")
                    for sub in range(2):
                        blk = slice(512 * sub, 512 * (sub + 1))
                        gcol = slice(1024 * k + 512 * sub, 1024 * k + 512 * sub + 512)
                        nc.tensor.matmul(yps[:, blk],
                                         AB[:, 128 * q:128 * (q + 1)],
                                         RC[0:2, gcol], start=True, stop=True)
                    nc.vector.tensor_tensor(out=ys, in0=xt[q][k],
                                            in1=yps, op=OP.add)
                    nc.sync.dma_start(
                        out=yout[128 * q:128 * (q + 1), 1024 * k:1024 * (k + 1)],
                        in_=ys,
                    )
    return nc


def host_prep(x, Wq, bq, Wk, bk, Wc, Wv, bv, gamma):
    """Fold weights on host; build per-core input maps."""
    x = np.asarray(x, dtype=np.float32)
    Wq = np.asarray(Wq, np.float32); bq = np.asarray(bq, np.float32)
    Wk = np.asarray(Wk, np.float32); bk = np.asarray(bk, np.float32)
    Wc = np.asarray(Wc, np.float32)
    Wv = np.asarray(Wv, np.float32); bv = np.asarray(bv, np.float32)
    gamma = float(np.asarray(gamma).reshape(-1)[0])

    wqv, wkv = Wc[:INTER], Wc[INTER:]
    wq_eff = (wqv @ Wq).astype(np.float32)          # [C]
    wk_eff = (wkv @ Wk).astype(np.float32)
    bq_eff = float(wqv @ bq)
    bk_eff = float(wkv @ bk)
    g = gamma / float(N)

    import ml_dtypes
    bf = ml_dtypes.bfloat16
    # wqk[p, q, 0] = wq_eff chunk q; wqk[p, q, 1] = wk_eff chunk q
    wqk_np = np.stack(
        [np.stack([wq_eff[:128], wk_eff[:128]], axis=1),
         np.stack([wq_eff[128:], wk_eff[128:]], axis=1)], axis=1).astype(bf)
    # wq_eff replicated along the output free dim for the direct-eqb matmul
    wqrep_np = np.broadcast_to(
        np.stack([wq_eff[:128], wq_eff[128:]], axis=1)[:, :, None].astype(bf),
        (128, 2, 128)).copy()
    # g folded into Wv so the s/V matmuls directly give g*s_nb, g*V_nb
    wvt_np = (g * Wv.T).reshape(2, 128, C).transpose(1, 0, 2).astype(bf)
    bvrow = bv.reshape(1, C)
    bvgn = (g * N * bv).reshape(1, C).astype(np.float32)

    xr = x.reshape(B, C, N)
    xbf_all = xr.astype(bf)
    in_maps = []
    for core in range(NCORES):
        b, half = core // 2, core % 2
        own = slice(HALF * half, HALF * (half + 1))
        other = slice(HALF * (1 - half), HALF * (2 - half))
        in_maps.append({
            "xin": np.ascontiguousarray(xr[b][:, own]),
            "xbh": np.ascontiguousarray(xbf_all[b][:, other]),
            "wqk": np.ascontiguousarray(wqk_np),
            "wqrep": np.ascontiguousarray(wqrep_np),
            "wvt": np.ascontiguousarray(wvt_np),
            "bvrow": np.ascontiguousarray(bvrow),
            "bvgn": np.ascontiguousarray(bvgn),
        })
    return in_maps, (g, bq_eff, bk_eff)


def assemble(results):
    """Stitch per-core halves into the full output [B, C, H, W]."""
    y = np.empty((B, C, N), dtype=np.float32)
    for core in range(NCORES):
        b, half = core // 2, core % 2
        y[b, :, HALF * half:HALF * (half + 1)] = results[core]["yout"]
    return y.reshape(B, C, H, W)


def kernel(**inputs):
    install_wait_split()
    in_maps, (g, bq_eff, bk_eff) = host_prep(**inputs)
    nc = build_kernel(g, bq_eff, bk_eff)
    res = run_bass_kernel_spmd(nc, in_maps, core_ids=list(range(NCORES)))
    return assemble(res.results)



# revision 33
# speedup vs baseline: 1.0340x; 1.0340x over previous
"""Trainium2 Bass kernel for Attention_concat (separable PAM attention).

Math (per batch b, N = H*W = 4096):
    eqn[n] = wq_eff . x[:, n]                  (wq_eff = Wq^T Wc[:64])
    ekn[m] = wk_eff . x[:, m]
    y[c, m] = x[c, m] + A[c] + Bv[c] * ekn[m]
with global reductions u = x @ 1, t = x @ eqn and
    Bv = g*Wv u + g*N*bv
    A  = g*Wv (t + (bq_eff+bk_eff) u) + bv*(g*E + g*N*(bq_eff+bk_eff))
    E  = wq_eff . u,   g = gamma / N

Precision: the attention correction is ~1.5e-4 of |y|, so the whole pipeline
runs in bf16 (x is loaded bf16, y stored bf16); worst-case rel-to-scale error
~8e-3 vs the 2e-2 gate.

Sharding: 2 cores per batch, each handles half the spatial columns. Each core
redundantly computes the global reductions over the full x[b] (own half + a
bf16 copy of the other half), then writes its own 2048 output columns.

Engine split: PE broadcasts eqn into PSUM (stationary wq_eff replicated along
the free dim) and computes the ekn rows; DVE does the t-reduction with fused
affine_mul_reduce; ACT accumulates u via activation-Copy accum_out; Pool
(gpsimd) takes overflow t-tiles and the ekn PSUM->SBUF row copies. The final
y = x + A + Bv*ekn is two accumulating matmuls per 512-block (rank-2 AB x RC
plus identity @ x), then PSUM->SBUF bf16 copies split DVE/ACT, 4 output DMAs.
The A/Bv rows are assembled entirely in partitions 0-1 via a [128,2]-stationary
matmul plus a per-partition scalar fold (no SBUF->SBUF DMA). Dummy matmuls on
a memset tile keep the PE p-state at full clock across idle windows.

Module-level workarounds (this container's walrus accepts only one sync-wait
per instruction): extra waits are hoisted onto single-wait NoOps at BIR level,
and the Tile tail drain is rebuilt the same way.
"""

import json as _json

import numpy as np

import concourse.bass as bass
import concourse.bass2jax as _b2j
import concourse.bass_utils as _bu
import concourse.mybir as mybir
import concourse.tile as tile
from concourse.bass_utils import run_bass_kernel_spmd
from concourse.vector_clock import ScopedClock, VectorClock

B, C, H, W = 4, 256, 64, 64
N = H * W            # 4096
INTER = C // 4       # 64
NCORES = 8
HALF = N // 2        # 2048 output columns per core
F32 = mybir.dt.float32
BF16 = mybir.dt.bfloat16
AX = mybir.AxisListType
OP = mybir.AluOpType
ACTF = mybir.ActivationFunctionType

# wpk free-dim layout (per q chunk): [0]=wq_eff col, [1]=wk_eff col,
# [2:130]=wq_eff replicated 128, [130:386]=g*Wv^T, [386:514]=identity (q=0)
WPK_COLS = 514
# rpk2 row-pack: [0:256]=bv, [256:512]=wk_eff, [512]=g*N, [513:1025]=ones
RPK_COLS = 1025


def _split_multi_waits(bir: dict) -> dict:
    """The nix walrus accepts only ONE sync-wait command per instruction.
    Hoist extra waits onto preceding single-wait NoOps on the same engine
    (sequencers execute in program order, so semantics are unchanged)."""
    ctr = 0
    for fn in bir.get("functions", []):
        for blk in fn.get("blocks", []):
            insts = blk.get("instructions")
            if not insts:
                continue
            out = []
            for inst in insts:
                si = inst.get("sync_info") or {}
                waits = si.get("on_wait") or []
                if len(waits) > 1 and inst.get("engine", "Unassigned") != "Unassigned":
                    for w in waits[:-1]:
                        ctr += 1
                        out.append({
                            "debug": inst.get("debug", 0),
                            "engine": inst["engine"],
                            "ins": [], "outs": [],
                            "name": f"{inst['name']}-ws{ctr}",
                            "opcode": "NoOp",
                            "sync_info": {"on_update": [], "on_wait": [w]},
                        })
                    si["on_wait"] = [waits[-1]]
                out.append(inst)
            blk["instructions"] = out
    return bir


_WAIT_SPLIT_DONE = False


def install_wait_split():
    global _WAIT_SPLIT_DONE
    if _WAIT_SPLIT_DONE:
        return
    orig = _bu.compile_bir_kernel

    def wrapped(bir_json, *a, **kw):
        d = _json.loads(bir_json)
        _split_multi_waits(d)
        return orig(_json.dumps(d).encode(), *a, **kw)

    _bu.compile_bir_kernel = wrapped
    _b2j.compile_bir_kernel = wrapped
    _WAIT_SPLIT_DONE = True


class SplitDrainTileContext(tile.TileContext):
    """Tail fix for the same 1-wait walrus limit: park the global-clock waits
    on single-wait Nops spread across all five engines (they wait in
    parallel), then a wait-free drain + the usual barrier/reset."""

    def _drain_and_barrier(self, tick_clock, wait_clock):
        gc = tick_clock.global_clock
        nprocs = len(gc)
        engines = [self.nc.sync, self.nc.vector, self.nc.scalar,
                   self.nc.gpsimd, self.nc.tensor]
        idx = 0
        for proc in range(nprocs):
            if gc[proc] > 0:
                eng = engines[idx % len(engines)]
                idx += 1
                nop = eng.nop(nofuse=True, hint=f"tail_wait_p{proc}")
                vc = VectorClock([0] * nprocs)
                vc.require_at_least(proc, gc[proc])
                wait_clock.add_sem_waits(nop.ins, ScopedClock({None: vc}))
        self.nc.sync.drain()
        self.nc.all_engine_barrier()
        assert self.sems is not None
        popped = self.nc._tile_sem_poison_stack.pop()
        assert popped is self._sem_poison
        self.nc.clear_and_free_semaphores(list(self.sems.allocated().values()))
        self.nc.all_engine_barrier()


def build_kernel(g: float, bq_eff: float, bk_eff: float):
    """Build the per-core Bass program. g = gamma/N."""
    bqk = bq_eff + bk_eff
    nc = bass.Bass()
    xo = nc.dram_tensor("xo", [128, 2, HALF], BF16, kind="ExternalInput")
    xr = nc.dram_tensor("xr", [128, 2, HALF], BF16, kind="ExternalInput")
    wpk = nc.dram_tensor("wpk", [128, 2, WPK_COLS], BF16, kind="ExternalInput")
    rpk2 = nc.dram_tensor("rpk2", [1, RPK_COLS], BF16, kind="ExternalInput")
    yout = nc.dram_tensor("yout", [128, 2, HALF], BF16, kind="ExternalOutput")

    with SplitDrainTileContext(nc) as tc:
        with (
            tc.tile_pool(name="persist", bufs=1) as pp,
            tc.tile_pool(name="trash", bufs=3) as tp,
            tc.tile_pool(name="ypool", bufs=4) as yp,
            tc.tile_pool(name="psm", bufs=3, space="PSUM") as psm,
            tc.tile_pool(name="pbig", bufs=2, space="PSUM") as pbig,
            tc.tile_pool(name="pwu", bufs=1, space="PSUM") as pwu,
        ):
            # --- persistent tiles -------------------------------------------
            xt = [[pp.tile([128, 2, 1024], BF16, tag=f"x{s}_{k}",
                           name=f"x{s}_{k}")
                   for k in range(2)] for s in range(2)]  # s=0 own, s=1 other
            wpk_sb = pp.tile([128, 2, WPK_COLS], BF16, tag="wpk")
            rpk2_sb = pp.tile([1, RPK_COLS], BF16, tag="rpk2")
            M2sb = pp.tile([128, 2 * C], BF16, tag="M2sb")  # I + Bv x wk_eff
            A_sb = pp.tile([1, C], BF16, tag="A_sb")
            Bv_sb = pp.tile([1, C], BF16, tag="Bv_sb")
            tacc = pp.tile([128, 2, 4], F32, tag="tacc")
            uacc = pp.tile([128, 2, 4], F32, tag="uacc")
            tu = pp.tile([128, 2, 2], F32, tag="tu")     # col0 u, col1 t+bqk*u
            tub = pp.tile([128, 2, 2], BF16, tag="tub")
            t2 = pp.tile([128, 2], F32, tag="t2")
            u2 = pp.tile([128, 2], F32, tag="u2")
            scsel = pp.tile([1, 1], BF16, tag="scsel")   # [sc] stationary
            wusrc = pp.tile([128, 512], BF16, tag="wusrc")
            atr = pp.tile([1, 1], BF16, tag="atr")       # ACT table-load dummy

            wqcol = lambda q: wpk_sb[:, q, 0:1]
            wkcol = lambda q: wpk_sb[:, q, 1:2]
            wqrep = lambda q: wpk_sb[:, q, 2:130]
            wvt = lambda q: wpk_sb[:, q, 130:386]
            ident = wpk_sb[:, 0, 386:WPK_COLS]
            bvrow = rpk2_sb[0:1, 0:C]
            wkrow = lambda q: rpk2_sb[0:1, C + 128 * q:C + 128 * (q + 1)]
            cgn = rpk2_sb[0:1, 2 * C:2 * C + 1]          # [g*N]
            ones512 = rpk2_sb[0:1, 2 * C + 1:2 * C + 513]

            # --- t=0: DMAs + cheap setup ------------------------------------
            # sync queue: own x halves first, then weights
            for k in range(2):
                nc.sync.dma_start(out=xt[0][k], in_=xo[:, :, 1024 * k:1024 * (k + 1)])
            nc.sync.dma_start(out=wpk_sb, in_=wpk[:, :, :])
            nc.sync.dma_start(out=rpk2_sb, in_=rpk2[:, :])
            # scalar (ACT hwdge) queue: other-half x
            for k in range(2):
                nc.scalar.dma_start(out=xt[1][k], in_=xr[:, :, 1024 * k:1024 * (k + 1)])

            nc.vector.memset(wusrc, 0.5)
            # ACT function-table load happens at the first activation: trigger
            # it early on a 1-element dummy so it overlaps the DMA wait.
            nc.scalar.activation(out=atr, in_=wusrc[0:1, 0:1], func=ACTF.Copy)

            # PE p-state ramp: dummy matmuls with no DMA dependency.
            def dummy_mm(n, tag):
                for i in range(n):
                    wu = pwu.tile([128, 512], F32, tag="wu", name=f"wu_{tag}_{i}")
                    nc.tensor.matmul(wu, wusrc[:, 0:128], wusrc,
                                     start=True, stop=True)

            dummy_mm(7, "pre")

            # --- phase A: stream x, eq broadcast, t/u reductions ------------
            for sb in range(4):
                s, k = sb // 2, sb % 2
                src = xt[s][k]
                # eq broadcast: [128, 1024] PSUM, 2 blocks x 2 q-chunks
                eqb = pbig.tile([128, 1024], F32, tag="big", name=f"eqb{sb}")
                for half in range(2):
                    blk = slice(512 * half, 512 * (half + 1))
                    for q in range(2):
                        nc.tensor.matmul(eqb[:, blk], wqrep(q), src[:, q, blk],
                                         start=(q == 0), stop=(q == 1))
                dummy_mm(3, f"a{sb}")
                # u accumulation on ACT
                for q in range(2):
                    trsh = tp.tile([128, 1024], BF16, tag="tr")
                    nc.scalar.activation(out=trsh, in_=src[:, q, :],
                                         func=ACTF.Copy,
                                         accum_out=uacc[:, q, sb:sb + 1])
                # t reduction: fused (eqb+0)*x with free-dim accumulate (DVE)
                for q in range(2):
                    trsh = tp.tile([128, 1024], BF16, tag="tr")
                    nc.vector.scalar_tensor_tensor(
                        out=trsh, in0=eqb, scalar=0.0, in1=src[:, q, :],
                        op0=OP.add, op1=OP.mult,
                        accum_out=tacc[:, q, sb:sb + 1])

            # --- tail: fold reductions into A/Bv rows -----------------------
            dummy_mm(4, "t0")
            for q in range(2):
                nc.vector.tensor_reduce(out=t2[:, q:q + 1], in_=tacc[:, q, :],
                                        axis=AX.X, op=OP.add)
                nc.vector.tensor_reduce(out=u2[:, q:q + 1], in_=uacc[:, q, :],
                                        axis=AX.X, op=OP.add)
                nc.vector.tensor_copy(out=tu[:, q, 0:1], in_=u2[:, q:q + 1])
                nc.vector.tensor_scalar(out=tu[:, q, 1:2], in0=u2[:, q:q + 1],
                                        scalar1=bqk, scalar2=None, op0=OP.mult)
                nc.vector.tensor_tensor(out=tu[:, q, 1:2], in0=tu[:, q, 1:2],
                                        in1=t2[:, q:q + 1], op=OP.add)
            nc.vector.tensor_copy(out=tub, in_=tu)

            # E = wq_eff . u -> sc = g*E + g*N*bqk into the [sc] stationary.
            ep = psm.tile([1, 1], F32, tag="sm", name="ep")
            for q in range(2):
                nc.tensor.matmul(ep, tub[:, q, 0:1], wqcol(q),
                                 start=(q == 0), stop=(q == 1))
            nc.scalar.activation(out=scsel, in_=ep, func=ACTF.Copy,
                                 scale=g, bias=g * N * bqk)
            # A row: g*Wv(t+bqk u) + sc*bv ;  Bv row: g*Wv u + g*N*bv
            # (each a [1, C] PSUM accumulation chain at partition 0)
            Pa = psm.tile([1, C], F32, tag="sm", name="Pa")
            Pb = psm.tile([1, C], F32, tag="sm", name="Pb")
            for q in range(2):
                nc.tensor.matmul(Pa, tub[:, q, 1:2], wvt(q),
                                 start=(q == 0), stop=False)
            nc.tensor.matmul(Pa, scsel, bvrow, start=False, stop=True)
            for q in range(2):
                nc.tensor.matmul(Pb, tub[:, q, 0:1], wvt(q),
                                 start=(q == 0), stop=False)
            nc.tensor.matmul(Pb, cgn, bvrow, start=False, stop=True)
            nc.vector.tensor_copy(out=A_sb, in_=Pa)
            nc.vector.tensor_copy(out=Bv_sb, in_=Pb)
            dummy_mm(2, "t1")
            # M2 = Bv x wk_eff outer product: M2ps[c', qj*C + ci] per qj chunk
            M2ps = pwu.tile([128, 2 * C], F32, tag="wu", name="M2ps")
            for qj in range(2):
                nc.tensor.matmul(M2ps[:, C * qj:C * (qj + 1)], wkrow(qj),
                                 Bv_sb, start=True, stop=True)
            # M2sb = M2ps (+ identity on the diagonal blocks), cast to bf16
            for qj in range(2):
                dcol = slice(C * qj + 128 * qj, C * qj + 128 * (qj + 1))
                ocol = slice(C * qj + 128 * (1 - qj), C * qj + 128 * (2 - qj))
                nc.vector.tensor_tensor(out=M2sb[:, dcol], in0=M2ps[:, dcol],
                                        in1=ident, op=OP.add)
                nc.vector.tensor_copy(out=M2sb[:, ocol], in_=M2ps[:, ocol])

            # --- phase C: y[ci,m] = sum_c' M2[c',ci] x[c',m] + A[ci] --------
            copy_eng = [nc.vector, nc.scalar, nc.vector, nc.scalar]
            dma_eng = [nc.sync, nc.scalar, nc.sync, nc.scalar]
            bi = 0
            for k in range(2):
                for q in range(2):
                    yps = pbig.tile([128, 1024], F32, tag="big",
                                    name=f"yps{q}_{k}")
                    for half in range(2):
                        blk = slice(512 * half, 512 * (half + 1))
                        for qj in range(2):
                            m2s = M2sb[:, C * qj + 128 * q:C * qj + 128 * (q + 1)]
                            nc.tensor.matmul(yps[:, blk], m2s,
                                             xt[0][k][:, qj, blk],
                                             start=(qj == 0), stop=False)
                        nc.tensor.matmul(yps[:, blk],
                                         A_sb[0:1, 128 * q:128 * (q + 1)],
                                         ones512, start=False, stop=True)
                    ysb = yp.tile([128, 1024], BF16, tag="y")
                    if copy_eng[bi] is nc.scalar:
                        nc.scalar.activation(out=ysb, in_=yps, func=ACTF.Copy)
                    else:
                        nc.vector.tensor_copy(out=ysb, in_=yps)
                    dma_eng[bi].dma_start(
                        out=yout[:, q, 1024 * k:1024 * (k + 1)], in_=ysb)
                    bi += 1
    return nc


def host_prep(x, Wq, bq, Wk, bk, Wc, Wv, bv, gamma):
    """Fold weights on host; build per-core input maps."""
    x = np.asarray(x, dtype=np.float32)
    Wq = np.asarray(Wq, np.float32); bq = np.asarray(bq, np.float32)
    Wk = np.asarray(Wk, np.float32); bk = np.asarray(bk, np.float32)
    Wc = np.asarray(Wc, np.float32)
    Wv = np.asarray(Wv, np.float32); bv = np.asarray(bv, np.float32)
    gamma = float(np.asarray(gamma).reshape(-1)[0])

    wqv, wkv = Wc[:INTER], Wc[INTER:]
    wq_eff = (wqv @ Wq).astype(np.float32)          # [C]
    wk_eff = (wkv @ Wk).astype(np.float32)
    bq_eff = float(wqv @ bq)
    bk_eff = float(wkv @ bk)
    g = gamma / float(N)

    import ml_dtypes
    bf = ml_dtypes.bfloat16

    wpk = np.zeros((128, 2, WPK_COLS), np.float32)
    for q in range(2):
        cs = slice(128 * q, 128 * (q + 1))
        wpk[:, q, 0] = wq_eff[cs]
        wpk[:, q, 1] = wk_eff[cs]
        wpk[:, q, 2:130] = wq_eff[cs][:, None]
        wpk[:, q, 130:386] = g * Wv.T[cs, :]
    wpk[:, 0, 386:WPK_COLS] = np.eye(128, dtype=np.float32)
    wpk = wpk.astype(bf)

    # rpk2 row-pack: bv ++ wk_eff ++ [g*N] ++ ones512
    rpk2 = np.concatenate(
        [bv, wk_eff, [g * N], np.ones(512, np.float32)]
    ).reshape(1, RPK_COLS).astype(bf)

    xr_all = x.reshape(B, C, N)
    xb = xr_all.astype(bf).reshape(B, 2, 128, N)     # [B, q, p, n]
    in_maps = []
    for core in range(NCORES):
        b, half = core // 2, core % 2
        own = slice(HALF * half, HALF * (half + 1))
        other = slice(HALF * (1 - half), HALF * (2 - half))
        in_maps.append({
            "xo": np.ascontiguousarray(xb[b][:, :, own].transpose(1, 0, 2)),
            "xr": np.ascontiguousarray(xb[b][:, :, other].transpose(1, 0, 2)),
            "wpk": np.ascontiguousarray(wpk),
            "rpk2": np.ascontiguousarray(rpk2),
        })
    return in_maps, (g, bq_eff, bk_eff)


def assemble(results):
    """Stitch per-core halves into the full output [B, C, H, W]."""
    y = np.empty((B, C, N), dtype=np.float32)
    for core in range(NCORES):
        b, half = core // 2, core % 2
        yo = np.asarray(results[core]["yout"], dtype=np.float32)  # [128,2,2048]
        y[b, :, HALF * half:HALF * (half + 1)] = \
            yo.transpose(1, 0, 2).reshape(C, HALF)
    return y.reshape(B, C, H, W)


def kernel(**inputs):
    install_wait_split()
    in_maps, (g, bq_eff, bk_eff) = host_prep(**inputs)
    nc = build_kernel(g, bq_eff, bk_eff)
    res = run_bass_kernel_spmd(nc, in_maps, core_ids=list(range(NCORES)))
    return assemble(res.results)


# revision 34
# speedup vs baseline: 1.2499x; 1.2088x over previous
"""Trainium2 Bass kernel for Attention_concat (separable PAM attention).

Math (per batch b, N = H*W = 4096):
    eqn[n] = wq_eff . x[:, n]                  (wq_eff = Wq^T Wc[:64])
    ekn[m] = wk_eff . x[:, m]
    y[c, m] = x[c, m] + A[c] + Bv[c] * ekn[m]
with global reductions u = x @ 1, t = x @ eqn and
    Bv = g*Wv u + g*N*bv
    A  = g*Wv (t + (bq_eff+bk_eff) u) + bv*(g*E + g*N*(bq_eff+bk_eff))
    E  = wq_eff . u,   g = gamma / N

Precision: the attention correction is ~1.5e-4 of |y|, so the whole pipeline
runs in bf16 (x is loaded bf16, y stored bf16); measured rel-to-scale error
~3e-3 vs the 2e-2 gate.

Sharding: 2 cores per batch, each handles half the spatial columns. Each core
redundantly computes the global reductions over the full x[b] (own half + a
bf16 copy of the other half), then writes its own 2048 output columns.

Engine/DMA split: inputs ride both HWDGE rings (sync: x own half; scalar: the
weight pack first, then x other half) so the eqb-gating weights land early.
PE broadcasts eqn into PSUM (stationary wq_eff replicated along the free dim)
and computes ekn rows; DVE does the t-reduction via scalar_tensor_tensor with
accum_out; ACT accumulates u via activation-Copy accum_out; ekn PSUM->RC
copies split ACT/DVE. The A/Bv rows are assembled in one [2,C] PSUM
accumulation chain (tub stationary + [gN,0]/[0,sc] selector rows against the
bv row) — no cross-partition moves. Phase C: rank-2 AB x RC matmul per
512-block; two blocks finish as DVE adds (x + psum), two as PE identity-fold
plus ACT copy, then 4 output DMAs alternating rings. Dummy matmuls keep the
PE p-state up across idle windows.

Module-level workarounds (this container's walrus accepts only one sync-wait
per instruction): extra waits are hoisted onto single-wait NoOps at BIR level,
and the Tile tail drain is rebuilt the same way.
"""

import json as _json

import numpy as np

import concourse.bass as bass
import concourse.bass2jax as _b2j
import concourse.bass_utils as _bu
import concourse.mybir as mybir
import concourse.tile as tile
from concourse.bass_utils import run_bass_kernel_spmd
from concourse.vector_clock import ScopedClock, VectorClock

B, C, H, W = 4, 256, 64, 64
N = H * W            # 4096
INTER = C // 4       # 64
NCORES = 8
HALF = N // 2        # 2048 output columns per core
F32 = mybir.dt.float32
BF16 = mybir.dt.bfloat16
AX = mybir.AxisListType
OP = mybir.AluOpType
ACTF = mybir.ActivationFunctionType

# wpk free-dim layout (per q chunk): [0]=wq_eff col, [1]=wk_eff col,
# [2:130]=wq_eff replicated 128, [130:386]=g*Wv^T, [386:514]=identity (q=0)
WPK_COLS = 514
# rpk2 row-pack: [0:256]=bv, [256]=g*N, [257]=0
RPK_COLS = 258


def _split_multi_waits(bir: dict) -> dict:
    """The nix walrus accepts only ONE sync-wait command per instruction.
    Hoist extra waits onto preceding single-wait NoOps on the same engine
    (sequencers execute in program order, so semantics are unchanged)."""
    ctr = 0
    for fn in bir.get("functions", []):
        for blk in fn.get("blocks", []):
            insts = blk.get("instructions")
            if not insts:
                continue
            out = []
            for inst in insts:
                si = inst.get("sync_info") or {}
                waits = si.get("on_wait") or []
                if len(waits) > 1 and inst.get("engine", "Unassigned") != "Unassigned":
                    for w in waits[:-1]:
                        ctr += 1
                        out.append({
                            "debug": inst.get("debug", 0),
                            "engine": inst["engine"],
                            "ins": [], "outs": [],
                            "name": f"{inst['name']}-ws{ctr}",
                            "opcode": "NoOp",
                            "sync_info": {"on_update": [], "on_wait": [w]},
                        })
                    si["on_wait"] = [waits[-1]]
                out.append(inst)
            blk["instructions"] = out
    return bir


_WAIT_SPLIT_DONE = False


def install_wait_split():
    global _WAIT_SPLIT_DONE
    if _WAIT_SPLIT_DONE:
        return
    orig = _bu.compile_bir_kernel

    def wrapped(bir_json, *a, **kw):
        d = _json.loads(bir_json)
        _split_multi_waits(d)
        return orig(_json.dumps(d).encode(), *a, **kw)

    _bu.compile_bir_kernel = wrapped
    _b2j.compile_bir_kernel = wrapped
    _WAIT_SPLIT_DONE = True


class SplitDrainTileContext(tile.TileContext):
    """Tail fix for the same 1-wait walrus limit: park the global-clock waits
    on single-wait Nops spread across all five engines (they wait in
    parallel), then a wait-free drain + the usual barrier/reset."""

    def _drain_and_barrier(self, tick_clock, wait_clock):
        gc = tick_clock.global_clock
        nprocs = len(gc)
        engines = [self.nc.sync, self.nc.vector, self.nc.scalar,
                   self.nc.gpsimd, self.nc.tensor]
        idx = 0
        for proc in range(nprocs):
            if gc[proc] > 0:
                eng = engines[idx % len(engines)]
                idx += 1
                nop = eng.nop(nofuse=True, hint=f"tail_wait_p{proc}")
                vc = VectorClock([0] * nprocs)
                vc.require_at_least(proc, gc[proc])
                wait_clock.add_sem_waits(nop.ins, ScopedClock({None: vc}))
        self.nc.sync.drain()
        self.nc.all_engine_barrier()
        assert self.sems is not None
        popped = self.nc._tile_sem_poison_stack.pop()
        assert popped is self._sem_poison
        self.nc.clear_and_free_semaphores(list(self.sems.allocated().values()))
        self.nc.all_engine_barrier()


def build_kernel(g: float, bq_eff: float, bk_eff: float):
    """Build the per-core Bass program. g = gamma/N."""
    bqk = bq_eff + bk_eff
    nc = bass.Bass()
    xo0 = nc.dram_tensor("xo0", [128, 2, 1024], BF16, kind="ExternalInput")
    xo1 = nc.dram_tensor("xo1", [128, 2, 1024], BF16, kind="ExternalInput")
    xr0 = nc.dram_tensor("xr0", [128, 2, 1024], BF16, kind="ExternalInput")
    xr1 = nc.dram_tensor("xr1", [128, 2, 1024], BF16, kind="ExternalInput")
    wpk = nc.dram_tensor("wpk", [128, 2, WPK_COLS], BF16, kind="ExternalInput")
    rpk2 = nc.dram_tensor("rpk2", [1, RPK_COLS], BF16, kind="ExternalInput")
    rones = nc.dram_tensor("rones", [1, HALF], BF16, kind="ExternalInput")
    yout = nc.dram_tensor("yout", [128, 2, HALF], BF16, kind="ExternalOutput")

    with SplitDrainTileContext(nc) as tc:
        with (
            tc.tile_pool(name="persist", bufs=1) as pp,
            tc.tile_pool(name="trash", bufs=3) as tp,
            tc.tile_pool(name="ypool", bufs=4) as yp,
            tc.tile_pool(name="psm", bufs=2, space="PSUM") as psm,
            tc.tile_pool(name="pbig", bufs=2, space="PSUM") as pbig,
            tc.tile_pool(name="pwu", bufs=1, space="PSUM") as pwu,
        ):
            # --- persistent tiles -------------------------------------------
            xt = [[pp.tile([128, 2, 1024], BF16, tag=f"x{s}_{k}",
                           name=f"x{s}_{k}")
                   for k in range(2)] for s in range(2)]  # s=0 own, s=1 other
            wpk_sb = pp.tile([128, 2, WPK_COLS], BF16, tag="wpk")
            rpk2_sb = pp.tile([1, RPK_COLS], BF16, tag="rpk2")
            RC = pp.tile([2, HALF], BF16, tag="RC")      # row0 ekn, row1 ones
            AB = pp.tile([2, C], BF16, tag="AB")         # row0 Bv, row1 A
            tacc = pp.tile([128, 2, 4], F32, tag="tacc")
            uacc = pp.tile([128, 2, 4], F32, tag="uacc")
            tu = pp.tile([128, 2, 2], F32, tag="tu")     # col0 u, col1 t+bqk*u
            tub = pp.tile([128, 2, 2], BF16, tag="tub")
            t2 = pp.tile([128, 2], F32, tag="t2")
            u2 = pp.tile([128, 2], F32, tag="u2")
            scsel = pp.tile([1, 2], BF16, tag="scsel")   # [0, sc] selector
            wusrc = pp.tile([128, 512], BF16, tag="wusrc")
            atr = pp.tile([1, 1], BF16, tag="atr")       # ACT table-load dummy

            wqcol = lambda q: wpk_sb[:, q, 0:1]
            wkcol = lambda q: wpk_sb[:, q, 1:2]
            wqrep = lambda q: wpk_sb[:, q, 2:130]
            wvt = lambda q: wpk_sb[:, q, 130:386]
            ident = wpk_sb[:, 0, 386:WPK_COLS]
            bvrow = rpk2_sb[0:1, 0:C]
            cgn = rpk2_sb[0:1, C:C + 2]                  # [g*N, 0]

            # --- t=0: DMAs + cheap setup ------------------------------------
            # sync ring: own x halves + small rows; scalar ring: weights
            # first (they gate eqb), then the other x half.
            nc.scalar.dma_start(out=wpk_sb, in_=wpk[:, :, :])
            nc.sync.dma_start(out=xt[0][0], in_=xo0[:, :, :])
            nc.sync.dma_start(out=xt[0][1], in_=xo1[:, :, :])
            nc.scalar.dma_start(out=xt[1][0], in_=xr0[:, :, :])
            nc.scalar.dma_start(out=xt[1][1], in_=xr1[:, :, :])
            nc.sync.dma_start(out=rpk2_sb, in_=rpk2[:, :])
            nc.sync.dma_start(out=RC[1:2, :], in_=rones[:, :])

            nc.vector.memset(wusrc, 0.5)
            nc.vector.memset(scsel, 0.0)
            # ACT function-table load happens at the first activation: trigger
            # it early on a 1-element dummy so it overlaps the DMA wait.
            nc.scalar.activation(out=atr, in_=wusrc[0:1, 0:1], func=ACTF.Copy)

            # PE p-state ramp: dummy matmuls with no DMA dependency.
            def dummy_mm(n, tag):
                for i in range(n):
                    wu = pwu.tile([128, 512], F32, tag="wu", name=f"wu_{tag}_{i}")
                    nc.tensor.matmul(wu, wusrc[:, 0:128], wusrc,
                                     start=True, stop=True)

            dummy_mm(6, "pre")

            # --- phase A: stream x, eq broadcast, t/u reductions, ekn -------
            for sb in range(4):
                s, k = sb // 2, sb % 2
                src = xt[s][k]
                # eq broadcast: [128, 1024] PSUM, 2 blocks x 2 q-chunks
                eqb = pbig.tile([128, 1024], F32, tag="big", name=f"eqb{sb}")
                for half in range(2):
                    blk = slice(512 * half, 512 * (half + 1))
                    for q in range(2):
                        nc.tensor.matmul(eqb[:, blk], wqrep(q), src[:, q, blk],
                                         start=(q == 0), stop=(q == 1))
                # ekn rows for own half: [1, 512] PSUM -> RC row0
                # (copies split ACT for sb0, DVE for sb1)
                if s == 0:
                    for half in range(2):
                        blk = slice(512 * half, 512 * (half + 1))
                        gcol = slice(1024 * k + 512 * half,
                                     1024 * k + 512 * half + 512)
                        ekp = psm.tile([1, 512], F32, tag="sm",
                                       name=f"ek{sb}_{half}")
                        for q in range(2):
                            nc.tensor.matmul(ekp, wkcol(q), src[:, q, blk],
                                             start=(q == 0), stop=(q == 1))
                        if k == 0:
                            nc.scalar.copy(out=RC[0:1, gcol], in_=ekp)
                        else:
                            nc.vector.tensor_copy(out=RC[0:1, gcol], in_=ekp)
                dummy_mm(2, f"a{sb}")
                # u accumulation on ACT
                for q in range(2):
                    trsh = tp.tile([128, 1024], BF16, tag="tr")
                    nc.scalar.activation(out=trsh, in_=src[:, q, :],
                                         func=ACTF.Copy,
                                         accum_out=uacc[:, q, sb:sb + 1])
                # t reduction: fused (eqb+0)*x with free-dim accumulate (DVE)
                for q in range(2):
                    trsh = tp.tile([128, 1024], BF16, tag="tr")
                    nc.vector.scalar_tensor_tensor(
                        out=trsh, in0=eqb, scalar=0.0, in1=src[:, q, :],
                        op0=OP.add, op1=OP.mult,
                        accum_out=tacc[:, q, sb:sb + 1])

            # --- tail: fold reductions into the AB rows ---------------------
            dummy_mm(3, "t0")
            for q in range(2):
                nc.vector.tensor_reduce(out=t2[:, q:q + 1], in_=tacc[:, q, :],
                                        axis=AX.X, op=OP.add)
                nc.vector.tensor_reduce(out=u2[:, q:q + 1], in_=uacc[:, q, :],
                                        axis=AX.X, op=OP.add)
                nc.vector.tensor_copy(out=tu[:, q, 0:1], in_=u2[:, q:q + 1])
                nc.vector.tensor_scalar(out=tu[:, q, 1:2], in0=u2[:, q:q + 1],
                                        scalar1=bqk, scalar2=None, op0=OP.mult)
                nc.vector.tensor_tensor(out=tu[:, q, 1:2], in0=tu[:, q, 1:2],
                                        in1=t2[:, q:q + 1], op=OP.add)
            nc.vector.tensor_copy(out=tub, in_=tu)

            # E = wq_eff . u -> sc = g*E + g*N*bqk into scsel = [0, sc]
            ep = psm.tile([1, 1], F32, tag="sm", name="ep")
            for q in range(2):
                nc.tensor.matmul(ep, tub[:, q, 0:1], wqcol(q),
                                 start=(q == 0), stop=(q == 1))
            nc.scalar.activation(out=scsel[0:1, 1:2], in_=ep, func=ACTF.Copy,
                                 scale=g, bias=g * N * bqk)
            # AB rows in one [2, C] PSUM accumulation chain:
            #   row0 (Bv) = g*Wv u        + g*N*bv + 0*bv
            #   row1 (A)  = g*Wv(t+bqk u) + 0      + sc*bv
            P = psm.tile([2, C], F32, tag="sm", name="P")
            for q in range(2):
                nc.tensor.matmul(P, tub[:, q, :], wvt(q),
                                 start=(q == 0), stop=False)
            nc.tensor.matmul(P, cgn, bvrow, start=False, stop=False)
            nc.tensor.matmul(P, scsel, bvrow, start=False, stop=True)
            dummy_mm(2, "t1")
            nc.vector.tensor_copy(out=AB, in_=P)

            # --- phase C: y = x + A + Bv*ekn over own half ------------------
            # blocks (k,q)=(0,0),(1,1): DVE add x+psum; (0,1),(1,0): PE
            # identity-fold + ACT copy.
            dma_eng = [nc.sync, nc.scalar, nc.sync, nc.scalar]
            bi = 0
            for k in range(2):
                for q in range(2):
                    on_dve = (k == q)
                    yps = pbig.tile([128, 1024], F32, tag="big",
                                    name=f"yps{q}_{k}")
                    for half in range(2):
                        blk = slice(512 * half, 512 * (half + 1))
                        gcol = slice(1024 * k + 512 * half,
                                     1024 * k + 512 * half + 512)
                        nc.tensor.matmul(yps[:, blk],
                                         AB[:, 128 * q:128 * (q + 1)],
                                         RC[0:2, gcol], start=True,
                                         stop=on_dve)
                        if not on_dve:
                            nc.tensor.matmul(yps[:, blk], ident,
                                             xt[0][k][:, q, blk],
                                             start=False, stop=True)
                    ysb = yp.tile([128, 1024], BF16, tag="y")
                    if on_dve:
                        nc.vector.tensor_tensor(out=ysb, in0=xt[0][k][:, q, :],
                                                in1=yps, op=OP.add)
                    else:
                        nc.scalar.activation(out=ysb, in_=yps, func=ACTF.Copy)
                    dma_eng[bi].dma_start(
                        out=yout[:, q, 1024 * k:1024 * (k + 1)], in_=ysb)
                    bi += 1
    return nc


def host_prep(x, Wq, bq, Wk, bk, Wc, Wv, bv, gamma):
    """Fold weights on host; build per-core input maps."""
    x = np.asarray(x, dtype=np.float32)
    Wq = np.asarray(Wq, np.float32); bq = np.asarray(bq, np.float32)
    Wk = np.asarray(Wk, np.float32); bk = np.asarray(bk, np.float32)
    Wc = np.asarray(Wc, np.float32)
    Wv = np.asarray(Wv, np.float32); bv = np.asarray(bv, np.float32)
    gamma = float(np.asarray(gamma).reshape(-1)[0])

    wqv, wkv = Wc[:INTER], Wc[INTER:]
    wq_eff = (wqv @ Wq).astype(np.float32)          # [C]
    wk_eff = (wkv @ Wk).astype(np.float32)
    bq_eff = float(wqv @ bq)
    bk_eff = float(wkv @ bk)
    g = gamma / float(N)

    import ml_dtypes
    bf = ml_dtypes.bfloat16

    wpk = np.zeros((128, 2, WPK_COLS), np.float32)
    for q in range(2):
        cs = slice(128 * q, 128 * (q + 1))
        wpk[:, q, 0] = wq_eff[cs]
        wpk[:, q, 1] = wk_eff[cs]
        wpk[:, q, 2:130] = wq_eff[cs][:, None]
        wpk[:, q, 130:386] = g * Wv.T[cs, :]
    wpk[:, 0, 386:WPK_COLS] = np.eye(128, dtype=np.float32)
    wpk = wpk.astype(bf)

    rpk2 = np.concatenate([bv, [g * N, 0.0]]).reshape(1, RPK_COLS).astype(bf)
    rones = np.ones((1, HALF), dtype=bf)

    xr_all = x.reshape(B, C, N)
    xb = xr_all.astype(bf).reshape(B, 2, 128, N)     # [B, q, p, n]
    in_maps = []
    for core in range(NCORES):
        b, half = core // 2, core % 2
        o0 = HALF * half
        r0 = HALF * (1 - half)
        in_maps.append({
            "xo0": np.ascontiguousarray(
                xb[b][:, :, o0:o0 + 1024].transpose(1, 0, 2)),
            "xo1": np.ascontiguousarray(
                xb[b][:, :, o0 + 1024:o0 + 2048].transpose(1, 0, 2)),
            "xr0": np.ascontiguousarray(
                xb[b][:, :, r0:r0 + 1024].transpose(1, 0, 2)),
            "xr1": np.ascontiguousarray(
                xb[b][:, :, r0 + 1024:r0 + 2048].transpose(1, 0, 2)),
            "wpk": np.ascontiguousarray(wpk),
            "rpk2": np.ascontiguousarray(rpk2),
            "rones": np.ascontiguousarray(rones),
        })
    return in_maps, (g, bq_eff, bk_eff)


def assemble(results):
    """Stitch per-core halves into the full output [B, C, H, W]."""
    y = np.empty((B, C, N), dtype=np.float32)
    for core in range(NCORES):
        b, half = core // 2, core % 2
        yo = np.asarray(results[core]["yout"], dtype=np.float32)  # [128,2,2048]
        y[b, :, HALF * half:HALF * (half + 1)] = \
            yo.transpose(1, 0, 2).reshape(C, HALF)
    return y.reshape(B, C, H, W)


def kernel(**inputs):
    install_wait_split()
    in_maps, (g, bq_eff, bk_eff) = host_prep(**inputs)
    nc = build_kernel(g, bq_eff, bk_eff)
    res = run_bass_kernel_spmd(nc, in_maps, core_ids=list(range(NCORES)))
    return assemble(res.results)


# revision 36
# speedup vs baseline: 1.2730x; 1.0185x over previous
"""Trainium2 Bass kernel for Attention_concat (separable PAM attention).

Math (per batch b, N = H*W = 4096):
    eqn[n] = wq_eff . x[:, n]                  (wq_eff = Wq^T Wc[:64])
    ekn[m] = wk_eff . x[:, m]
    y[c, m] = x[c, m] + A[c] + Bv[c] * ekn[m]
with global reductions u = x @ 1, t = x @ eqn and
    Bv = g*Wv u + g*N*bv
    A  = g*Wv (t + (bq_eff+bk_eff) u) + bv*(g*E + g*N*(bq_eff+bk_eff))
    E  = wq_eff . u,   g = gamma / N

Precision: the attention correction is ~1.5e-4 of |y|, so the whole pipeline
runs in bf16 (x is loaded bf16, y stored bf16); measured rel-to-scale error
~3e-3 vs the 2e-2 gate.

Sharding: 2 cores per batch, each handles half the spatial columns. Each core
redundantly computes the global reductions over the full x[b] (own half + a
bf16 copy of the other half), then writes its own 2048 output columns.

Engine/DMA split: inputs ride both HWDGE rings (sync: x own half; scalar: the
weight pack first, then x other half) so the eqb-gating weights land early.
PE broadcasts eqn into PSUM (stationary wq_eff replicated along the free dim)
and computes ekn rows; DVE does the t-reduction via scalar_tensor_tensor with
accum_out; ACT accumulates u via activation-Copy accum_out; ekn PSUM->RC
copies split ACT/DVE. The A/Bv rows are assembled in one [2,C] PSUM
accumulation chain (tub stationary + [gN,0]/[0,sc] selector rows against the
bv row) — no cross-partition moves. Phase C: rank-2 AB x RC matmul per
512-block; two blocks finish as DVE adds (x + psum), two as PE identity-fold
plus ACT copy, then 4 output DMAs alternating rings. Dummy matmuls keep the
PE p-state up across idle windows.

Module-level workarounds (this container's walrus accepts only one sync-wait
per instruction): extra waits are hoisted onto single-wait NoOps at BIR level,
and the Tile tail drain is rebuilt the same way.
"""

import json as _json

import numpy as np

import concourse.bass as bass
import concourse.bass2jax as _b2j
import concourse.bass_utils as _bu
import concourse.mybir as mybir
import concourse.tile as tile
from concourse.bass_utils import run_bass_kernel_spmd
from concourse.vector_clock import ScopedClock, VectorClock

B, C, H, W = 4, 256, 64, 64
N = H * W            # 4096
INTER = C // 4       # 64
NCORES = 8
HALF = N // 2        # 2048 output columns per core
F32 = mybir.dt.float32
BF16 = mybir.dt.bfloat16
AX = mybir.AxisListType
OP = mybir.AluOpType
ACTF = mybir.ActivationFunctionType

# wpka free-dim layout (per q chunk): [0]=wq_eff col, [1]=wk_eff col,
# [2:130]=wq_eff replicated 128  (gates phase A -> lands first)
WPKA_COLS = 130
# wpkb: [0:256]=g*Wv^T, [256:384]=identity in q=0  (tail-only -> lands last)
WPKB_COLS = 384
# rpk2 row-pack: [0:256]=bv, [256]=g*N, [257]=0
RPK_COLS = 258


def _split_multi_waits(bir: dict) -> dict:
    """The nix walrus accepts only ONE sync-wait command per instruction.
    Hoist extra waits onto preceding single-wait NoOps on the same engine
    (sequencers execute in program order, so semantics are unchanged)."""
    ctr = 0
    for fn in bir.get("functions", []):
        for blk in fn.get("blocks", []):
            insts = blk.get("instructions")
            if not insts:
                continue
            out = []
            for inst in insts:
                si = inst.get("sync_info") or {}
                waits = si.get("on_wait") or []
                if len(waits) > 1 and inst.get("engine", "Unassigned") != "Unassigned":
                    for w in waits[:-1]:
                        ctr += 1
                        out.append({
                            "debug": inst.get("debug", 0),
                            "engine": inst["engine"],
                            "ins": [], "outs": [],
                            "name": f"{inst['name']}-ws{ctr}",
                            "opcode": "NoOp",
                            "sync_info": {"on_update": [], "on_wait": [w]},
                        })
                    si["on_wait"] = [waits[-1]]
                out.append(inst)
            blk["instructions"] = out
    return bir


_WAIT_SPLIT_DONE = False


def install_wait_split():
    global _WAIT_SPLIT_DONE
    if _WAIT_SPLIT_DONE:
        return
    orig = _bu.compile_bir_kernel

    def wrapped(bir_json, *a, **kw):
        d = _json.loads(bir_json)
        _split_multi_waits(d)
        return orig(_json.dumps(d).encode(), *a, **kw)

    _bu.compile_bir_kernel = wrapped
    _b2j.compile_bir_kernel = wrapped
    _WAIT_SPLIT_DONE = True


class SplitDrainTileContext(tile.TileContext):
    """Tail fix for the same 1-wait walrus limit: park the global-clock waits
    on single-wait Nops spread across all five engines (they wait in
    parallel), then a wait-free drain + the usual barrier/reset."""

    def _drain_and_barrier(self, tick_clock, wait_clock):
        gc = tick_clock.global_clock
        nprocs = len(gc)
        engines = [self.nc.sync, self.nc.vector, self.nc.scalar,
                   self.nc.gpsimd, self.nc.tensor]
        idx = 0
        for proc in range(nprocs):
            if gc[proc] > 0:
                eng = engines[idx % len(engines)]
                idx += 1
                nop = eng.nop(nofuse=True, hint=f"tail_wait_p{proc}")
                vc = VectorClock([0] * nprocs)
                vc.require_at_least(proc, gc[proc])
                wait_clock.add_sem_waits(nop.ins, ScopedClock({None: vc}))
        self.nc.sync.drain()
        self.nc.all_engine_barrier()
        assert self.sems is not None
        popped = self.nc._tile_sem_poison_stack.pop()
        assert popped is self._sem_poison
        self.nc.clear_and_free_semaphores(list(self.sems.allocated().values()))
        self.nc.all_engine_barrier()


def build_kernel(g: float, bq_eff: float, bk_eff: float):
    """Build the per-core Bass program. g = gamma/N."""
    bqk = bq_eff + bk_eff
    nc = bass.Bass()
    xo = nc.dram_tensor("xo", [128, 2, HALF], BF16, kind="ExternalInput")
    xr = nc.dram_tensor("xr", [128, 2, HALF], BF16, kind="ExternalInput")
    wpka = nc.dram_tensor("wpka", [128, 2, WPKA_COLS], BF16, kind="ExternalInput")
    wpkb = nc.dram_tensor("wpkb", [128, 2, WPKB_COLS], BF16, kind="ExternalInput")
    rpk2 = nc.dram_tensor("rpk2", [1, RPK_COLS], BF16, kind="ExternalInput")
    rones = nc.dram_tensor("rones", [1, HALF], BF16, kind="ExternalInput")
    yout = nc.dram_tensor("yout", [128, 2, HALF], BF16, kind="ExternalOutput")

    with SplitDrainTileContext(nc) as tc:
        with (
            tc.tile_pool(name="persist", bufs=1) as pp,
            tc.tile_pool(name="trasha", bufs=1) as tpa,
            tc.tile_pool(name="trashd", bufs=1) as tpd,
            tc.tile_pool(name="ypool", bufs=4) as yp,
            tc.tile_pool(name="psm", bufs=2, space="PSUM") as psm,
            tc.tile_pool(name="pbig", bufs=2, space="PSUM") as pbig,
            tc.tile_pool(name="pwu", bufs=1, space="PSUM") as pwu,
        ):
            # --- persistent tiles -------------------------------------------
            xts = [pp.tile([128, 2, HALF], BF16, tag=f"x{s}", name=f"x{s}")
                   for s in range(2)]                    # s=0 own, s=1 other
            xt = [[xts[s][:, :, 1024 * k:1024 * (k + 1)] for k in range(2)]
                  for s in range(2)]
            wpka_sb = pp.tile([128, 2, WPKA_COLS], BF16, tag="wpka")
            wpkb_sb = pp.tile([128, 2, WPKB_COLS], BF16, tag="wpkb")
            rpk2_sb = pp.tile([1, RPK_COLS], BF16, tag="rpk2")
            RC = pp.tile([2, HALF], BF16, tag="RC")      # row0 ekn, row1 ones
            AB = pp.tile([2, C], BF16, tag="AB")         # row0 Bv, row1 A
            tacc = pp.tile([128, 2, 4], F32, tag="tacc")
            uacc = pp.tile([128, 2, 4], F32, tag="uacc")
            tu = pp.tile([128, 2, 2], F32, tag="tu")     # col0 u, col1 t+bqk*u
            tub = pp.tile([128, 2, 2], BF16, tag="tub")
            t2 = pp.tile([128, 2], F32, tag="t2")
            u2 = pp.tile([128, 2], F32, tag="u2")
            scsel = pp.tile([1, 2], BF16, tag="scsel")   # [0, sc] selector
            wusrc = pp.tile([128, 512], BF16, tag="wusrc")
            atr = pp.tile([1, 1], BF16, tag="atr")       # ACT table-load dummy

            wqcol = lambda q: wpka_sb[:, q, 0:1]
            wkcol = lambda q: wpka_sb[:, q, 1:2]
            wqrep = lambda q: wpka_sb[:, q, 2:130]
            wvt = lambda q: wpkb_sb[:, q, 0:256]
            ident = wpkb_sb[:, 0, 256:WPKB_COLS]
            bvrow = rpk2_sb[0:1, 0:C]
            cgn = rpk2_sb[0:1, C:C + 2]                  # [g*N, 0]

            # --- t=0: DMAs + cheap setup ------------------------------------
            # sync ring: own x (one 1-MiB transfer) + small rows; scalar
            # ring: phase-A weights first, the other x half, then the
            # tail-only weights.
            nc.scalar.dma_start(out=wpka_sb, in_=wpka[:, :, :])
            nc.sync.dma_start(out=xts[0], in_=xo[:, :, :])
            nc.scalar.dma_start(out=xts[1], in_=xr[:, :, :])
            nc.sync.dma_start(out=rpk2_sb, in_=rpk2[:, :])
            nc.sync.dma_start(out=RC[1:2, :], in_=rones[:, :])
            nc.scalar.dma_start(out=wpkb_sb, in_=wpkb[:, :, :])

            nc.vector.memset(wusrc, 0.5)
            nc.vector.memset(scsel, 0.0)
            # ACT function-table load happens at the first activation: trigger
            # it early on a 1-element dummy so it overlaps the DMA wait.
            nc.scalar.activation(out=atr, in_=wusrc[0:1, 0:1], func=ACTF.Copy)

            # PE p-state ramp: dummy matmuls with no DMA dependency.
            def dummy_mm(n, tag):
                for i in range(n):
                    wu = pwu.tile([128, 512], F32, tag="wu", name=f"wu_{tag}_{i}")
                    nc.tensor.matmul(wu, wusrc[:, 0:128], wusrc,
                                     start=True, stop=True)

            dummy_mm(9, "pre")

            # --- phase A: stream x, eq broadcast, t/u reductions, ekn -------
            for sb in range(4):
                s, k = sb // 2, sb % 2
                src = xt[s][k]
                # eq broadcast: [128, 1024] PSUM, 2 blocks x 2 q-chunks
                eqb = pbig.tile([128, 1024], F32, tag="big", name=f"eqb{sb}")
                for half in range(2):
                    blk = slice(512 * half, 512 * (half + 1))
                    for q in range(2):
                        nc.tensor.matmul(eqb[:, blk], wqrep(q), src[:, q, blk],
                                         start=(q == 0), stop=(q == 1))
                # ekn rows for own half: [1, 512] PSUM (copies come after
                # the u/t stream below so they don't block the engine queues)
                eks = []
                if s == 0:
                    for half in range(2):
                        blk = slice(512 * half, 512 * (half + 1))
                        gcol = slice(1024 * k + 512 * half,
                                     1024 * k + 512 * half + 512)
                        ekp = psm.tile([1, 512], F32, tag="sm",
                                       name=f"ek{sb}_{half}")
                        for q in range(2):
                            nc.tensor.matmul(ekp, wkcol(q), src[:, q, blk],
                                             start=(q == 0), stop=(q == 1))
                        eks.append((ekp, gcol))
                dummy_mm(2, f"a{sb}")
                # u accumulation on ACT
                for q in range(2):
                    trsh = tpa.tile([128, 1024], BF16, tag="tr")
                    nc.scalar.activation(out=trsh, in_=src[:, q, :],
                                         func=ACTF.Copy,
                                         accum_out=uacc[:, q, sb:sb + 1])
                # t reduction: fused (eqb+0)*x with free-dim accumulate (DVE)
                for q in range(2):
                    trsh = tpd.tile([128, 1024], BF16, tag="tr")
                    nc.vector.scalar_tensor_tensor(
                        out=trsh, in0=eqb, scalar=0.0, in1=src[:, q, :],
                        op0=OP.add, op1=OP.mult,
                        accum_out=tacc[:, q, sb:sb + 1])
                # ekn PSUM -> RC row0: ACT for sb0, DVE for sb1
                for ekp, gcol in eks:
                    if k == 0:
                        nc.scalar.copy(out=RC[0:1, gcol], in_=ekp)
                    else:
                        nc.vector.tensor_copy(out=RC[0:1, gcol], in_=ekp)

            # --- tail: fold reductions into the AB rows ---------------------
            dummy_mm(16, "t0")
            nc.vector.tensor_reduce(out=t2, in_=tacc, axis=AX.X, op=OP.add)
            nc.vector.tensor_reduce(out=u2, in_=uacc, axis=AX.X, op=OP.add)
            nc.vector.tensor_copy(out=tu[:, :, 0], in_=u2)
            nc.vector.tensor_scalar(out=tu[:, :, 1], in0=u2,
                                    scalar1=bqk, scalar2=None, op0=OP.mult)
            nc.vector.tensor_tensor(out=tu[:, :, 1], in0=tu[:, :, 1],
                                    in1=t2, op=OP.add)
            nc.vector.tensor_copy(out=tub, in_=tu)

            # E = wq_eff . u -> sc = g*E + g*N*bqk into scsel = [0, sc]
            ep = psm.tile([1, 1], F32, tag="sm", name="ep")
            for q in range(2):
                nc.tensor.matmul(ep, tub[:, q, 0:1], wqcol(q),
                                 start=(q == 0), stop=(q == 1))
            nc.scalar.activation(out=scsel[0:1, 1:2], in_=ep, func=ACTF.Copy,
                                 scale=g, bias=g * N * bqk)
            # AB rows in one [2, C] PSUM accumulation chain:
            #   row0 (Bv) = g*Wv u        + g*N*bv + 0*bv
            #   row1 (A)  = g*Wv(t+bqk u) + 0      + sc*bv
            P = psm.tile([2, C], F32, tag="sm", name="P")
            for q in range(2):
                nc.tensor.matmul(P, tub[:, q, :], wvt(q),
                                 start=(q == 0), stop=False)
            nc.tensor.matmul(P, cgn, bvrow, start=False, stop=False)
            nc.tensor.matmul(P, scsel, bvrow, start=False, stop=True)
            dummy_mm(2, "t1")
            nc.vector.tensor_copy(out=AB, in_=P)

            # --- phase C: y = x + A + Bv*ekn over own half ------------------
            # blocks (k,q)=(0,0),(1,1): DVE add x+psum; (0,1),(1,0): PE
            # identity-fold + ACT copy.
            dma_eng = [nc.sync, nc.scalar, nc.sync, nc.scalar]
            bi = 0
            for k in range(2):
                for q in range(2):
                    on_dve = (k == q)
                    yps = pbig.tile([128, 1024], F32, tag="big",
                                    name=f"yps{q}_{k}")
                    for half in range(2):
                        blk = slice(512 * half, 512 * (half + 1))
                        gcol = slice(1024 * k + 512 * half,
                                     1024 * k + 512 * half + 512)
                        nc.tensor.matmul(yps[:, blk],
                                         AB[:, 128 * q:128 * (q + 1)],
                                         RC[0:2, gcol], start=True,
                                         stop=on_dve)
                        if not on_dve:
                            nc.tensor.matmul(yps[:, blk], ident,
                                             xt[0][k][:, q, blk],
                                             start=False, stop=True)
                    ysb = yp.tile([128, 1024], BF16, tag="y")
                    if on_dve:
                        nc.vector.tensor_tensor(out=ysb, in0=xt[0][k][:, q, :],
                                                in1=yps, op=OP.add)
                    else:
                        nc.scalar.activation(out=ysb, in_=yps, func=ACTF.Copy)
                    dma_eng[bi].dma_start(
                        out=yout[:, q, 1024 * k:1024 * (k + 1)], in_=ysb)
                    bi += 1
    return nc


def host_prep(x, Wq, bq, Wk, bk, Wc, Wv, bv, gamma):
    """Fold weights on host; build per-core input maps."""
    x = np.asarray(x, dtype=np.float32)
    Wq = np.asarray(Wq, np.float32); bq = np.asarray(bq, np.float32)
    Wk = np.asarray(Wk, np.float32); bk = np.asarray(bk, np.float32)
    Wc = np.asarray(Wc, np.float32)
    Wv = np.asarray(Wv, np.float32); bv = np.asarray(bv, np.float32)
    gamma = float(np.asarray(gamma).reshape(-1)[0])

    wqv, wkv = Wc[:INTER], Wc[INTER:]
    wq_eff = (wqv @ Wq).astype(np.float32)          # [C]
    wk_eff = (wkv @ Wk).astype(np.float32)
    bq_eff = float(wqv @ bq)
    bk_eff = float(wkv @ bk)
    g = gamma / float(N)

    import ml_dtypes
    bf = ml_dtypes.bfloat16

    wpka = np.zeros((128, 2, WPKA_COLS), np.float32)
    wpkb = np.zeros((128, 2, WPKB_COLS), np.float32)
    for q in range(2):
        cs = slice(128 * q, 128 * (q + 1))
        wpka[:, q, 0] = wq_eff[cs]
        wpka[:, q, 1] = wk_eff[cs]
        wpka[:, q, 2:130] = wq_eff[cs][:, None]
        wpkb[:, q, 0:256] = g * Wv.T[cs, :]
    wpkb[:, 0, 256:WPKB_COLS] = np.eye(128, dtype=np.float32)
    wpka = wpka.astype(bf)
    wpkb = wpkb.astype(bf)

    rpk2 = np.concatenate([bv, [g * N, 0.0]]).reshape(1, RPK_COLS).astype(bf)
    rones = np.ones((1, HALF), dtype=bf)

    xr_all = x.reshape(B, C, N)
    xb = xr_all.astype(bf).reshape(B, 2, 128, N)     # [B, q, p, n]
    in_maps = []
    for core in range(NCORES):
        b, half = core // 2, core % 2
        own = slice(HALF * half, HALF * (half + 1))
        other = slice(HALF * (1 - half), HALF * (2 - half))
        in_maps.append({
            "xo": np.ascontiguousarray(xb[b][:, :, own].transpose(1, 0, 2)),
            "xr": np.ascontiguousarray(xb[b][:, :, other].transpose(1, 0, 2)),
            "wpka": np.ascontiguousarray(wpka),
            "wpkb": np.ascontiguousarray(wpkb),
            "rpk2": np.ascontiguousarray(rpk2),
            "rones": np.ascontiguousarray(rones),
        })
    return in_maps, (g, bq_eff, bk_eff)


def assemble(results):
    """Stitch per-core halves into the full output [B, C, H, W]."""
    y = np.empty((B, C, N), dtype=np.float32)
    for core in range(NCORES):
        b, half = core // 2, core % 2
        yo = np.asarray(results[core]["yout"], dtype=np.float32)  # [128,2,2048]
        y[b, :, HALF * half:HALF * (half + 1)] = \
            yo.transpose(1, 0, 2).reshape(C, HALF)
    return y.reshape(B, C, H, W)


def kernel(**inputs):
    install_wait_split()
    in_maps, (g, bq_eff, bk_eff) = host_prep(**inputs)
    nc = build_kernel(g, bq_eff, bk_eff)
    res = run_bass_kernel_spmd(nc, in_maps, core_ids=list(range(NCORES)))
    return assemble(res.results)


# revision 38
# speedup vs baseline: 1.3119x; 1.0306x over previous
"""Trainium2 Bass kernel for Attention_concat (separable PAM attention).

Math (per batch b, N = H*W = 4096):
    eqn[n] = wq_eff . x[:, n]                  (wq_eff = Wq^T Wc[:64])
    ekn[m] = wk_eff . x[:, m]
    y[c, m] = x[c, m] + A[c] + Bv[c] * ekn[m]
with global reductions u = x @ 1, t = x @ eqn and
    Bv = g*Wv u + g*N*bv
    A  = g*Wv (t + (bq_eff+bk_eff) u) + bv*(g*E + g*N*(bq_eff+bk_eff))
    E  = wq_eff . u,   g = gamma / N

Precision: the attention correction is ~1.5e-4 of |y|, so the whole pipeline
runs in bf16 (x is loaded bf16, y stored bf16); measured rel-to-scale error
~3e-3 vs the 2e-2 gate.

Sharding: 2 cores per batch, each handles half the spatial columns. Each core
redundantly computes the global reductions over the full x[b] (own half + a
bf16 copy of the other half), then writes its own 2048 output columns.

Engine/DMA split: inputs ride both HWDGE rings (sync: x own half; scalar: the
weight pack first, then x other half) so the eqb-gating weights land early.
PE broadcasts eqn into PSUM (stationary wq_eff replicated along the free dim)
and computes ekn rows; DVE does the t-reduction via scalar_tensor_tensor with
accum_out; ACT accumulates u via activation-Copy accum_out; ekn PSUM->RC
copies split ACT/DVE. The A/Bv rows are assembled in one [2,C] PSUM
accumulation chain (tub stationary + [gN,0]/[0,sc] selector rows against the
bv row) — no cross-partition moves. Phase C: rank-2 AB x RC matmul per
512-block; two blocks finish as DVE adds (x + psum), two as PE identity-fold
plus ACT copy, then 4 output DMAs alternating rings. Dummy matmuls keep the
PE p-state up across idle windows.

Module-level workarounds (this container's walrus accepts only one sync-wait
per instruction): extra waits are hoisted onto single-wait NoOps at BIR level,
and the Tile tail drain is rebuilt the same way.
"""

import json as _json

import numpy as np

import concourse.bass as bass
import concourse.bass2jax as _b2j
import concourse.bass_utils as _bu
import concourse.mybir as mybir
import concourse.tile as tile
from concourse.bass_utils import run_bass_kernel_spmd
from concourse.vector_clock import ScopedClock, VectorClock

B, C, H, W = 4, 256, 64, 64
N = H * W            # 4096
INTER = C // 4       # 64
NCORES = 8
HALF = N // 2        # 2048 output columns per core
F32 = mybir.dt.float32
BF16 = mybir.dt.bfloat16
AX = mybir.AxisListType
OP = mybir.AluOpType
ACTF = mybir.ActivationFunctionType

# wpka free-dim layout (per q chunk): [0]=wq_eff col, [1]=wk_eff col,
# [2:130]=wq_eff replicated 128  (gates phase A -> lands first)
WPKA_COLS = 130
# wpkb: [0:256]=g*Wv^T, [256:384]=identity in q=0  (tail-only -> lands last)
WPKB_COLS = 384
# rpk2 row-pack: [0:256]=bv, [256]=g*N, [257]=0
RPK_COLS = 258


def _split_multi_waits(bir: dict) -> dict:
    """The nix walrus accepts only ONE sync-wait command per instruction.
    Hoist extra waits onto preceding single-wait NoOps on the same engine
    (sequencers execute in program order, so semantics are unchanged)."""
    ctr = 0
    for fn in bir.get("functions", []):
        for blk in fn.get("blocks", []):
            insts = blk.get("instructions")
            if not insts:
                continue
            out = []
            for inst in insts:
                si = inst.get("sync_info") or {}
                waits = si.get("on_wait") or []
                if len(waits) > 1 and inst.get("engine", "Unassigned") != "Unassigned":
                    for w in waits[:-1]:
                        ctr += 1
                        out.append({
                            "debug": inst.get("debug", 0),
                            "engine": inst["engine"],
                            "ins": [], "outs": [],
                            "name": f"{inst['name']}-ws{ctr}",
                            "opcode": "NoOp",
                            "sync_info": {"on_update": [], "on_wait": [w]},
                        })
                    si["on_wait"] = [waits[-1]]
                out.append(inst)
            blk["instructions"] = out
    return bir


_WAIT_SPLIT_DONE = False


def install_wait_split():
    global _WAIT_SPLIT_DONE
    if _WAIT_SPLIT_DONE:
        return
    orig = _bu.compile_bir_kernel

    def wrapped(bir_json, *a, **kw):
        d = _json.loads(bir_json)
        _split_multi_waits(d)
        return orig(_json.dumps(d).encode(), *a, **kw)

    _bu.compile_bir_kernel = wrapped
    _b2j.compile_bir_kernel = wrapped
    _WAIT_SPLIT_DONE = True


class SplitDrainTileContext(tile.TileContext):
    """Tail fix for the same 1-wait walrus limit: park the global-clock waits
    on single-wait Nops spread across all five engines (they wait in
    parallel), then a wait-free drain + the usual barrier/reset."""

    def _drain_and_barrier(self, tick_clock, wait_clock):
        gc = tick_clock.global_clock
        nprocs = len(gc)
        engines = [self.nc.sync, self.nc.vector, self.nc.scalar,
                   self.nc.gpsimd, self.nc.tensor]
        idx = 0
        for proc in range(nprocs):
            if gc[proc] > 0:
                eng = engines[idx % len(engines)]
                idx += 1
                nop = eng.nop(nofuse=True, hint=f"tail_wait_p{proc}")
                vc = VectorClock([0] * nprocs)
                vc.require_at_least(proc, gc[proc])
                wait_clock.add_sem_waits(nop.ins, ScopedClock({None: vc}))
        self.nc.sync.drain()
        self.nc.all_engine_barrier()
        assert self.sems is not None
        popped = self.nc._tile_sem_poison_stack.pop()
        assert popped is self._sem_poison
        self.nc.clear_and_free_semaphores(list(self.sems.allocated().values()))
        self.nc.all_engine_barrier()


def build_kernel(g: float, bq_eff: float, bk_eff: float):
    """Build the per-core Bass program. g = gamma/N."""
    bqk = bq_eff + bk_eff
    nc = bass.Bass()
    xd = [[nc.dram_tensor(f"x{s}{k}", [128, 2, 1024], BF16,
                          kind="ExternalInput")
           for k in range(2)] for s in range(2)]
    wpka = nc.dram_tensor("wpka", [128, 2, WPKA_COLS], BF16, kind="ExternalInput")
    wpkb = nc.dram_tensor("wpkb", [128, 2, WPKB_COLS], BF16, kind="ExternalInput")
    rpk2 = nc.dram_tensor("rpk2", [1, RPK_COLS], BF16, kind="ExternalInput")
    rones = nc.dram_tensor("rones", [1, HALF], BF16, kind="ExternalInput")
    yout = nc.dram_tensor("yout", [128, 2, HALF], BF16, kind="ExternalOutput")

    with SplitDrainTileContext(nc) as tc:
        with (
            tc.tile_pool(name="persist", bufs=1) as pp,
            tc.tile_pool(name="trasha", bufs=1) as tpa,
            tc.tile_pool(name="trashd", bufs=1) as tpd,
            tc.tile_pool(name="ypool", bufs=4) as yp,
            tc.tile_pool(name="psm", bufs=2, space="PSUM") as psm,
            tc.tile_pool(name="pbig", bufs=2, space="PSUM") as pbig,
            tc.tile_pool(name="pwu", bufs=1, space="PSUM") as pwu,
        ):
            # --- persistent tiles -------------------------------------------
            xt = [[pp.tile([128, 2, 1024], BF16, tag=f"x{s}_{k}",
                           name=f"x{s}_{k}")
                   for k in range(2)] for s in range(2)]  # s=0 own, s=1 other
            wpka_sb = pp.tile([128, 2, WPKA_COLS], BF16, tag="wpka")
            wpkb_sb = pp.tile([128, 2, WPKB_COLS], BF16, tag="wpkb")
            rpk2_sb = pp.tile([1, RPK_COLS], BF16, tag="rpk2")
            RC = pp.tile([2, HALF], BF16, tag="RC")      # row0 ekn, row1 ones
            AB = pp.tile([2, C], BF16, tag="AB")         # row0 Bv, row1 A
            tacc = pp.tile([128, 2, 4], F32, tag="tacc")
            uacc = pp.tile([128, 2, 4], F32, tag="uacc")
            tu = pp.tile([128, 2, 2], F32, tag="tu")     # col0 u, col1 t+bqk*u
            tub = pp.tile([128, 2, 2], BF16, tag="tub")
            u2b = pp.tile([128, 2], BF16, tag="u2b")
            t2 = pp.tile([128, 2], F32, tag="t2")
            u2 = pp.tile([128, 2], F32, tag="u2")
            scsel = pp.tile([1, 2], BF16, tag="scsel")   # [0, sc] selector
            wusrc = pp.tile([128, 512], BF16, tag="wusrc")
            atr = pp.tile([1, 1], BF16, tag="atr")       # ACT table-load dummy

            wqcol = lambda q: wpka_sb[:, q, 0:1]
            wkcol = lambda q: wpka_sb[:, q, 1:2]
            wqrep = lambda q: wpka_sb[:, q, 2:130]
            wvt = lambda q: wpkb_sb[:, q, 0:256]
            ident = wpkb_sb[:, 0, 256:WPKB_COLS]
            bvrow = rpk2_sb[0:1, 0:C]
            cgn = rpk2_sb[0:1, C:C + 2]                  # [g*N, 0]

            # --- t=0: DMAs + cheap setup ------------------------------------
            # sync ring: the four x chunks in consumption order (FIFO per
            # ring, so each lands as the previous finishes); scalar ring:
            # weights + small rows (wpka gates the first eqb).
            nc.scalar.dma_start(out=wpka_sb, in_=wpka[:, :, :])
            for s in range(2):
                for k in range(2):
                    nc.sync.dma_start(out=xt[s][k], in_=xd[s][k][:, :, :])
            nc.scalar.dma_start(out=wpkb_sb, in_=wpkb[:, :, :])
            nc.scalar.dma_start(out=rpk2_sb, in_=rpk2[:, :])
            nc.scalar.dma_start(out=RC[1:2, :], in_=rones[:, :])

            nc.vector.memset(wusrc, 0.5)
            nc.vector.memset(scsel, 0.0)
            # ACT function-table load happens at the first activation: trigger
            # it early on a 1-element dummy so it overlaps the DMA wait.
            nc.scalar.activation(out=atr, in_=wusrc[0:1, 0:1], func=ACTF.Copy)

            # PE p-state ramp: dummy matmuls with no DMA dependency.
            def dummy_mm(n, tag):
                for i in range(n):
                    wu = pwu.tile([128, 512], F32, tag="wu", name=f"wu_{tag}_{i}")
                    nc.tensor.matmul(wu, wusrc[:, 0:128], wusrc,
                                     start=True, stop=True)

            dummy_mm(4, "pre")

            # --- phase A: stream x, eq broadcast, t/u reductions, ekn -------
            for sb in range(4):
                s, k = sb // 2, sb % 2
                src = xt[s][k]
                # eq broadcast: [128, 1024] PSUM, 2 blocks x 2 q-chunks
                eqb = pbig.tile([128, 1024], F32, tag="big", name=f"eqb{sb}")
                for half in range(2):
                    blk = slice(512 * half, 512 * (half + 1))
                    for q in range(2):
                        nc.tensor.matmul(eqb[:, blk], wqrep(q), src[:, q, blk],
                                         start=(q == 0), stop=(q == 1))
                # ekn rows for own half: [1, 512] PSUM (copies come after
                # the u/t stream below so they don't block the engine queues)
                eks = []
                if s == 0:
                    for half in range(2):
                        blk = slice(512 * half, 512 * (half + 1))
                        gcol = slice(1024 * k + 512 * half,
                                     1024 * k + 512 * half + 512)
                        ekp = psm.tile([1, 512], F32, tag="sm",
                                       name=f"ek{sb}_{half}")
                        for q in range(2):
                            nc.tensor.matmul(ekp, wkcol(q), src[:, q, blk],
                                             start=(q == 0), stop=(q == 1))
                        eks.append((ekp, gcol))
                dummy_mm(2, f"a{sb}")
                # u accumulation on ACT
                for q in range(2):
                    trsh = tpa.tile([128, 1024], BF16, tag="tr")
                    nc.scalar.activation(out=trsh, in_=src[:, q, :],
                                         func=ACTF.Copy,
                                         accum_out=uacc[:, q, sb:sb + 1])
                # t reduction: fused (eqb+0)*x with free-dim accumulate (DVE)
                for q in range(2):
                    trsh = tpd.tile([128, 1024], BF16, tag="tr")
                    nc.vector.scalar_tensor_tensor(
                        out=trsh, in0=eqb, scalar=0.0, in1=src[:, q, :],
                        op0=OP.add, op1=OP.mult,
                        accum_out=tacc[:, q, sb:sb + 1])
                # ekn PSUM -> RC row0: ACT for sb0, DVE for sb1
                for ekp, gcol in eks:
                    if k == 0:
                        nc.scalar.copy(out=RC[0:1, gcol], in_=ekp)
                    else:
                        nc.vector.tensor_copy(out=RC[0:1, gcol], in_=ekp)

            # --- tail: fold reductions into the AB rows ---------------------
            dummy_mm(6, "t0")
            nc.vector.tensor_reduce(out=u2, in_=uacc, axis=AX.X, op=OP.add)
            nc.vector.tensor_copy(out=u2b, in_=u2)
            nc.vector.tensor_reduce(out=t2, in_=tacc, axis=AX.X, op=OP.add)
            nc.vector.tensor_copy(out=tu[:, :, 0], in_=u2)
            nc.vector.tensor_scalar(out=tu[:, :, 1], in0=u2,
                                    scalar1=bqk, scalar2=None, op0=OP.mult)
            nc.vector.tensor_tensor(out=tu[:, :, 1], in0=tu[:, :, 1],
                                    in1=t2, op=OP.add)
            nc.vector.tensor_copy(out=tub, in_=tu)

            # E = wq_eff . u -> sc = g*E + g*N*bqk into scsel = [0, sc]
            # (runs off u2b so it overlaps the t folds above)
            ep = psm.tile([1, 1], F32, tag="sm", name="ep")
            for q in range(2):
                nc.tensor.matmul(ep, u2b[:, q:q + 1], wqcol(q),
                                 start=(q == 0), stop=(q == 1))
            nc.scalar.activation(out=scsel[0:1, 1:2], in_=ep, func=ACTF.Copy,
                                 scale=g, bias=g * N * bqk)
            # AB rows in one [2, C] PSUM accumulation chain:
            #   row0 (Bv) = g*Wv u        + g*N*bv + 0*bv
            #   row1 (A)  = g*Wv(t+bqk u) + 0      + sc*bv
            P = psm.tile([2, C], F32, tag="sm", name="P")
            for q in range(2):
                nc.tensor.matmul(P, tub[:, q, :], wvt(q),
                                 start=(q == 0), stop=False)
            nc.tensor.matmul(P, cgn, bvrow, start=False, stop=False)
            nc.tensor.matmul(P, scsel, bvrow, start=False, stop=True)
            dummy_mm(2, "t1")
            nc.vector.tensor_copy(out=AB, in_=P)

            # --- phase C: y = x + A + Bv*ekn over own half ------------------
            # blocks (k,q)=(0,0),(1,1): DVE add x+psum; (0,1),(1,0): PE
            # identity-fold + ACT copy.
            dma_eng = [nc.sync, nc.scalar, nc.sync, nc.scalar]
            bi = 0
            for k in range(2):
                for q in range(2):
                    on_dve = (bi % 2 == 0)
                    yps = pbig.tile([128, 1024], F32, tag="big",
                                    name=f"yps{q}_{k}")
                    for half in range(2):
                        blk = slice(512 * half, 512 * (half + 1))
                        gcol = slice(1024 * k + 512 * half,
                                     1024 * k + 512 * half + 512)
                        nc.tensor.matmul(yps[:, blk],
                                         AB[:, 128 * q:128 * (q + 1)],
                                         RC[0:2, gcol], start=True,
                                         stop=on_dve)
                        if not on_dve:
                            nc.tensor.matmul(yps[:, blk], ident,
                                             xt[0][k][:, q, blk],
                                             start=False, stop=True)
                    ysb = yp.tile([128, 1024], BF16, tag="y")
                    if on_dve:
                        nc.vector.tensor_tensor(out=ysb, in0=xt[0][k][:, q, :],
                                                in1=yps, op=OP.add)
                    else:
                        nc.scalar.activation(out=ysb, in_=yps, func=ACTF.Copy)
                    dma_eng[bi].dma_start(
                        out=yout[:, q, 1024 * k:1024 * (k + 1)], in_=ysb)
                    bi += 1
    return nc


def host_prep(x, Wq, bq, Wk, bk, Wc, Wv, bv, gamma):
    """Fold weights on host; build per-core input maps."""
    x = np.asarray(x, dtype=np.float32)
    Wq = np.asarray(Wq, np.float32); bq = np.asarray(bq, np.float32)
    Wk = np.asarray(Wk, np.float32); bk = np.asarray(bk, np.float32)
    Wc = np.asarray(Wc, np.float32)
    Wv = np.asarray(Wv, np.float32); bv = np.asarray(bv, np.float32)
    gamma = float(np.asarray(gamma).reshape(-1)[0])

    wqv, wkv = Wc[:INTER], Wc[INTER:]
    wq_eff = (wqv @ Wq).astype(np.float32)          # [C]
    wk_eff = (wkv @ Wk).astype(np.float32)
    bq_eff = float(wqv @ bq)
    bk_eff = float(wkv @ bk)
    g = gamma / float(N)

    import ml_dtypes
    bf = ml_dtypes.bfloat16

    wpka = np.zeros((128, 2, WPKA_COLS), np.float32)
    wpkb = np.zeros((128, 2, WPKB_COLS), np.float32)
    for q in range(2):
        cs = slice(128 * q, 128 * (q + 1))
        wpka[:, q, 0] = wq_eff[cs]
        wpka[:, q, 1] = wk_eff[cs]
        wpka[:, q, 2:130] = wq_eff[cs][:, None]
        wpkb[:, q, 0:256] = g * Wv.T[cs, :]
    wpkb[:, 0, 256:WPKB_COLS] = np.eye(128, dtype=np.float32)
    wpka = wpka.astype(bf)
    wpkb = wpkb.astype(bf)

    rpk2 = np.concatenate([bv, [g * N, 0.0]]).reshape(1, RPK_COLS).astype(bf)
    rones = np.ones((1, HALF), dtype=bf)

    xr_all = x.reshape(B, C, N)
    xb = xr_all.astype(bf).reshape(B, 2, 128, N)     # [B, q, p, n]
    in_maps = []
    for core in range(NCORES):
        b, half = core // 2, core % 2
        own = slice(HALF * half, HALF * (half + 1))
        other = slice(HALF * (1 - half), HALF * (2 - half))
        im = {
            "wpka": np.ascontiguousarray(wpka),
            "wpkb": np.ascontiguousarray(wpkb),
            "rpk2": np.ascontiguousarray(rpk2),
            "rones": np.ascontiguousarray(rones),
        }
        for s, sl in enumerate([own, other]):
            xs = xb[b][:, :, sl].transpose(1, 0, 2)
            for k in range(2):
                im[f"x{s}{k}"] = np.ascontiguousarray(
                    xs[:, :, 1024 * k:1024 * (k + 1)])
        in_maps.append(im)
    return in_maps, (g, bq_eff, bk_eff)


def assemble(results):
    """Stitch per-core halves into the full output [B, C, H, W]."""
    y = np.empty((B, C, N), dtype=np.float32)
    for core in range(NCORES):
        b, half = core // 2, core % 2
        yo = np.asarray(results[core]["yout"], dtype=np.float32)  # [128,2,2048]
        y[b, :, HALF * half:HALF * (half + 1)] = \
            yo.transpose(1, 0, 2).reshape(C, HALF)
    return y.reshape(B, C, H, W)


def kernel(**inputs):
    install_wait_split()
    in_maps, (g, bq_eff, bk_eff) = host_prep(**inputs)
    nc = build_kernel(g, bq_eff, bk_eff)
    res = run_bass_kernel_spmd(nc, in_maps, core_ids=list(range(NCORES)))
    return assemble(res.results)
